# revision 7
# baseline (speedup 1.0000x reference)
"""Trainium2 Bass kernel for CompositionalFC (moe_routing).

Reference computation:
    z[n,b,o] = x[b,i] @ weight[n,i,o] + bias[n,o]
    out[b,o] = relu( sum_n comp_weight[b,n] * z[n,b,o] )

Strategy: data-parallel over batch across 8 NeuronCores (512 rows each,
weight/bias replicated). Per core, for each expert n the partial product
z_n = x @ W_n is accumulated in PSUM from bf16 matmuls (lhsT = x^T tiles
stationary, W_n streaming), then combined into fp32 SBUF accumulators with
a single fused DVE op  acc = z*c[:,n] + acc  (comp_weight stays fp32).
The bias term sum_n c[b,n]*bias[n,o] == (comp_weight @ bias) seeds the
accumulators via a small K=16 fp32 matmul. ReLU on the way out.
"""

import sys

for _p in ("/opt/trn_rl_repo",):
    if _p not in sys.path:
        sys.path.insert(0, _p)

from contextlib import ExitStack

import ml_dtypes
import numpy as np

import concourse.bass as bass
import concourse.mybir as mybir
import concourse.tile as tile
from concourse import bacc
from concourse.bass_utils import run_bass_kernel_spmd

N_CORES = 8
BATCH, IN_DIM, OUT_DIM, N_EXP = 4096, 1024, 1024, 16
BS = BATCH // N_CORES          # 512 batch rows per core
P = 128                        # partitions
BT = BS // P                   # 4 batch tiles per core
KT = IN_DIM // P               # 8 contraction tiles per expert
FD = 512                       # matmul free dim / PSUM bank width (fp32)
NO = OUT_DIM // FD             # 2 output column tiles

F32 = mybir.dt.float32
BF16 = mybir.dt.bfloat16


def _build_kernel():
    nc = bacc.Bacc(
        "TRN2",
        target_bir_lowering=False,
        debug=False,
        num_devices=N_CORES,
    )
    xT = nc.declare_dram_parameter("xT", [IN_DIM, BS], BF16, isOutput=False)
    w = nc.declare_dram_parameter("w", [N_EXP, IN_DIM, OUT_DIM], BF16, isOutput=False)
    c = nc.declare_dram_parameter("c", [BS, N_EXP], F32, isOutput=False)
    cT = nc.declare_dram_parameter("cT", [N_EXP, BS], F32, isOutput=False)
    bias = nc.declare_dram_parameter("bias", [N_EXP, OUT_DIM], F32, isOutput=False)
    out = nc.declare_dram_parameter("out", [BS, OUT_DIM], F32, isOutput=True)

    with ExitStack() as ctx:
        tc = ctx.enter_context(tile.TileContext(nc))
        const = ctx.enter_context(tc.tile_pool(name="const", bufs=1))
        accp = ctx.enter_context(tc.tile_pool(name="accp", bufs=1))
        wpool = ctx.enter_context(tc.tile_pool(name="wpool", bufs=3))
        psum = ctx.enter_context(tc.tile_pool(name="psum", bufs=4, space="PSUM"))

        # --- persistent SBUF state -------------------------------------
        # Walrus allows a single HW wait slot per Matmult/Ldweights, so every
        # tile the PE reads is bounced DMA -> raw -> (DVE copy) -> consumed
        # tile: all PE waits then land on the one DVE semaphore and merge.
        xT_raw = const.tile([P, KT, BS], BF16, tag="xT_raw")
        nc.sync.dma_start(xT_raw[:], xT[:, :].rearrange("(kt p) b -> p kt b", p=P))
        xT_sb = const.tile([P, KT, BS], BF16, tag="xT_sb")
        nc.vector.tensor_copy(xT_sb[:], xT_raw[:])
        c_sb = const.tile([P, BT, N_EXP], F32, tag="c_sb")
        nc.sync.dma_start(c_sb[:], c[:, :].rearrange("(bt p) n -> p bt n", p=P))
        cT_raw = const.tile([N_EXP, BS], F32, tag="cT_raw")
        nc.sync.dma_start(cT_raw[:], cT[:, :])
        cT_sb = const.tile([N_EXP, BS], F32, tag="cT_sb")
        nc.vector.tensor_copy(cT_sb[:], cT_raw[:])
        bias_raw = const.tile([N_EXP, OUT_DIM], F32, tag="bias_raw")
        nc.sync.dma_start(bias_raw[:], bias[:, :])
        bias_sb = const.tile([N_EXP, OUT_DIM], F32, tag="bias_sb")
        nc.vector.tensor_copy(bias_sb[:], bias_raw[:])

        acc = [
            [
                accp.tile([P, FD], F32, name=f"acc_{bt}_{ot}", tag=f"acc_{bt}_{ot}")
                for ot in range(NO)
            ]
            for bt in range(BT)
        ]

        # --- seed accumulators with the bias term: acc = c @ bias ------
        for bt in range(BT):
            for ot in range(NO):
                pt = psum.tile([P, FD], F32, name="pt_init")
                nc.tensor.matmul(
                    pt[:],
                    lhsT=cT_sb[:, bt * P : (bt + 1) * P],
                    rhs=bias_sb[:, ot * FD : (ot + 1) * FD],
                    start=True,
                    stop=True,
                )
                nc.vector.tensor_copy(acc[bt][ot][:], pt[:])

        # --- main expert loop ------------------------------------------
        for n in range(N_EXP):
            w_raw = wpool.tile([P, KT, OUT_DIM], BF16, name="w_raw", tag="w_raw")
            nc.sync.dma_start(w_raw[:], w[n, :, :].rearrange("(kt p) o -> p kt o", p=P))
            w_sb = wpool.tile([P, KT, OUT_DIM], BF16, name="w_sb", tag="w_sb", bufs=2)
            nc.vector.tensor_copy(w_sb[:], w_raw[:])
            for bt in range(BT):
                zp = [psum.tile([P, FD], F32, name="zp") for _ in range(NO)]
                for kt_i in range(KT):
                    for ot in range(NO):
                        nc.tensor.matmul(
                            zp[ot][:],
                            lhsT=xT_sb[:, kt_i, bt * P : (bt + 1) * P],
                            rhs=w_sb[:, kt_i, ot * FD : (ot + 1) * FD],
                            start=(kt_i == 0),
                            stop=(kt_i == KT - 1),
                        )
                for ot in range(NO):
                    # acc += z * c[:, n]   (fused on DVE; c per-partition scalar)
                    nc.vector.scalar_tensor_tensor(
                        out=acc[bt][ot][:],
                        in0=zp[ot][:],
                        scalar=c_sb[:, bt, n : n + 1],
                        in1=acc[bt][ot][:],
                        op0=mybir.AluOpType.mult,
                        op1=mybir.AluOpType.add,
                    )

        # --- epilogue: relu + store ------------------------------------
        out_r = out[:, :].rearrange("(bt p) o -> p bt o", p=P)
        for bt in range(BT):
            for ot in range(NO):
                nc.scalar.activation(
                    acc[bt][ot][:], acc[bt][ot][:], mybir.ActivationFunctionType.Relu
                )
                nc.sync.dma_start(
                    out_r[:, bt, ot * FD : (ot + 1) * FD], acc[bt][ot][:]
                )

    nc.compile()
    return nc


_NC_CACHE = {}


def _get_nc():
    if "nc" not in _NC_CACHE:
        _NC_CACHE["nc"] = _build_kernel()
    return _NC_CACHE["nc"]


def _run(x, comp_weight, weight, bias, trace=False):
    x = np.ascontiguousarray(np.asarray(x, dtype=np.float32))
    comp_weight = np.ascontiguousarray(np.asarray(comp_weight, dtype=np.float32))
    weight = np.asarray(weight, dtype=np.float32)
    bias = np.ascontiguousarray(np.asarray(bias, dtype=np.float32))

    w_bf = np.ascontiguousarray(weight.astype(ml_dtypes.bfloat16))
    in_maps = []
    for r in range(N_CORES):
        sl = slice(r * BS, (r + 1) * BS)
        xs = x[sl]
        cs = comp_weight[sl]
        in_maps.append(
            {
                "xT": np.ascontiguousarray(xs.T).astype(ml_dtypes.bfloat16),
                "w": w_bf,
                "c": cs,
                "cT": np.ascontiguousarray(cs.T),
                "bias": bias,
            }
        )
    res = run_bass_kernel_spmd(
        _get_nc(), in_maps, core_ids=list(range(N_CORES)), trace=trace
    )
    out = np.concatenate([res.results[r]["out"] for r in range(N_CORES)], axis=0)
    return out, res


def kernel(x, comp_weight, weight, bias):
    out, _ = _run(x, comp_weight, weight, bias)
    return out
